# revision 19
# baseline (speedup 1.0000x reference)
"""Trainium2 Bass kernel for per-sample weight-demodulated 3x3 conv + leaky ReLU.

Problem (hardcoded shapes):
  input_vector: (8, 256, 128, 128) f32
  style_vector: (8, 256) f32
  weight:       (256, 256, 3, 3) f32
  out:          (8, 256, 128, 128) f32

Math (faithful to reference):
  ws[b,o,i,kh,kw] = weight[o,i,kh,kw] * style[b,i]
  demod[b,kw]     = rsqrt(sum_{o,i,kh} ws^2 + 1e-8)        # NOTE: sum excludes kw
  y[b] = leaky_relu(conv2d_same(x[b], ws[b]*demod), 0.2)

Sharding: data-parallel over batch, one sample per NeuronCore (8 cores).

Per-core kernel: 1D Winograd F(2,3) along the kw axis (kh stays direct),
cutting PE matmul work 1.5x vs the direct conv while keeping the PE the
dominant engine (the HAM clock manager holds full clock only under a
sustained-busy engine; a balanced multi-engine design oscillates at k=4).

  - weight prep: w DMA'd f16, PE-transposed, ACT style-scaled into wfin
    f16, demod per kw via ACT Square+accum and a ones-matmul broadcast.
  - the F(2,3) kernel transform G = [[1,0,0],[.5,.5,.5],[.5,-.5,.5],
    [0,0,1]] folds with demod into per-partition coefficients
    cu[u,kw] = 64*G*demod (u=2 negated, see below); U[u,kh] built by ACT
    copy-scale + DVE scalar_tensor_tensor chains, f16.  The x64 scale
    keeps U out of f16-denormal range; compensated exactly by scale=1/64
    inside the final ACT Prelu.
  - input transform: the F(2,3) data components are EXACTLY strided views
    of three contiguous helper tensors per (piece, cin-half):
        A_c = x_c - x_{c+2}   -> d0 = A[0::2],  d3 = A[1::2]
        T_c = x_{c+1}+x_{c+2} -> d1 = T[0::2]
        D_c = x_{c+1}-x_{c+2} -> d2 = -D[0::2]  (sign folded into cu[2])
    so DVE does just 3 packed-f16 tensor ops per (piece, cin-half) and the
    PE matmuls read the strided views directly.
  - conv: per 8-row chunk and cout-half, 24 f16 matmuls (4 u-comps x 3 kh
    x 2 cin-halves, free = 8 rows x 64 tiles = 512) accumulate into one
    [128,4,8,64] f32 PSUM tile (4 banks, double-buffered = all 8; the PE
    warmup ramp and the demod broadcast borrow tiles from the same pool).
  - epilogue: ACT copies m1,m2 from PSUM; DVE: y[2t] = (m1+m2)+m0,
    y[2t+1] = (m1-m2)-m3 (4 ops); ACT applies leaky-ReLU natively
    (parametric_relu alpha=0.2, scale=1/64, same ACT function set as
    Copy/Square/Sqrt - no table swaps) casting f16; one merged y DMA per
    chunk on the SP queue.
  - x lives in ONE big padded SBUF tile written once by disjoint per-piece
    sub-DMAs (no buffer reuse -> no DMA WAR races; overlap preserved).
"""

import numpy as np

B, CIN, COUT, K, H, W = 8, 256, 256, 3, 128, 128
P = 128
KB = CIN // P   # cin partition blocks   = 2
MB = COUT // P  # cout partition blocks  = 2
T = K * K       # taps = 9
NU = 4          # winograd F(2,3) components
NT = W // 2     # winograd tiles per row = 64
WP = W + 2      # padded row width = 130
NP = 8          # x pieces
PROWS = H // NP           # output rows per piece = 16
PPAD = PROWS + 2          # padded rows per piece = 18
CH_ROWS = 8               # output rows per psum chunk
CHUNKS = PROWS // CH_ROWS  # = 2
N_WARMUP = 40
USCALE = 64.0

# F(2,3) kernel transform G, rows u=0..3 over kw=0..2; u=2 sign-flipped
# because its data component is read as D = -(d2 view).
G_ROWS = [
    [1.0, 0.0, 0.0],
    [0.5, 0.5, 0.5],
    [0.5, -0.5, 0.5],
    [0.0, 0.0, 1.0],
]
U_SIGN = [1.0, 1.0, -1.0, 1.0]

_CACHE = {}


def _build(stage="full"):
    import concourse.mybir as mybir
    import concourse.tile as tile
    from concourse import bacc
    from concourse.masks import make_identity

    f32 = mybir.dt.float32
    f16 = mybir.dt.float16
    Alu = mybir.AluOpType
    Act = mybir.ActivationFunctionType

    nc = bacc.Bacc(None, target_bir_lowering=False)
    x_d = nc.dram_tensor("x", [CIN, H, W], f16, kind="ExternalInput")
    s_d = nc.dram_tensor("style", [1, CIN], f32, kind="ExternalInput")
    w_d = nc.dram_tensor("w", [COUT, CIN, K, K], f16, kind="ExternalInput")
    y_d = nc.dram_tensor("y", [COUT, H, W], f16, kind="ExternalOutput")

    y_flat = y_d[:].rearrange("o h w -> o (h w)")      # [256, 16384]
    y_pmf = y_d[:].rearrange("(m p) h w -> p m (h w)", p=P)  # [128, 2, 16384]
    w_flat = w_d[:].rearrange("o i kh kw -> o (i kh kw)")  # [256, 2304]

    with tile.TileContext(nc) as tc:
        with (
            tc.tile_pool(name="const", bufs=1) as const,
            tc.tile_pool(name="wtmp", bufs=1) as wtmp,
            tc.tile_pool(name="utmp", bufs=2) as utmp,
            tc.tile_pool(name="xbuf", bufs=1) as xbuf,
            tc.tile_pool(name="ttmp", bufs=2) as ttmp,
            tc.tile_pool(name="ctmp", bufs=2) as ctmp,
            tc.tile_pool(name="outp", bufs=2) as outp,
            tc.tile_pool(name="psum", bufs=2, space="PSUM") as psum,
        ):
            # ---------- constants ----------
            ident = const.tile([P, P], f16)
            make_identity(nc, ident)
            ones = const.tile([P, P], f32)
            nc.vector.memset(ones, 1.0)
            # dummy sqrt so the ACT function-set containing Sqrt (and Copy/
            # Square/parametric_relu) loads now, not mid-prep
            nc.scalar.sqrt(ones[0:1, 0:1], ones[0:1, 0:1])

            # ---------- weight load: very first DMA on the SP queue ----------
            wbuf = wtmp.tile([P, MB, CIN * T], f16)
            nc.sync.dma_start(
                out=wbuf[:],
                in_=w_flat.rearrange("(m p) f -> p m f", p=P),
            )

            # style per-partition: stile[p, kb] = style[kb*128 + p]
            stile = const.tile([P, KB], f32)
            for kb in range(KB):
                nc.sync.dma_start(
                    out=stile[:, kb : kb + 1],
                    in_=s_d[:].rearrange("one c -> c one")[kb * P : (kb + 1) * P, :],
                )

            # ---------- x: one big padded tile, disjoint piece sub-DMAs ------
            xq = xbuf.tile([P, KB, H + 2, WP], f16, name="xq")
            for kb in range(KB):
                nc.gpsimd.memset(xq[:, kb, :, 0], 0.0)
                nc.gpsimd.memset(xq[:, kb, :, WP - 1], 0.0)
                nc.gpsimd.memset(xq[:, kb, 0, :], 0.0)
                nc.gpsimd.memset(xq[:, kb, H + 1, :], 0.0)
            for p in range(NP):
                r_lo = p * PROWS
                for kb in range(KB):
                    nc.sync.dma_start(
                        out=xq[:, kb, r_lo + 1 : r_lo + 1 + PROWS, 1 : 1 + W],
                        in_=x_d[kb * P : (kb + 1) * P, r_lo : r_lo + PROWS, :],
                    )

            # ---------- PE warmup: ramp the clock while the w DMA flies ------
            # (borrows conv-psum tiles: matmul of ident into the f32 tile)
            for _ in range(N_WARMUP):
                gate = psum.tile([P, NU, CH_ROWS, NT], f32, name="pc")
                nc.tensor.matmul(
                    gate[:, 0, 0:2, :], ident, ident, start=True, stop=True
                )

            # ---------- weight prep, kw-major ----------
            wfin = const.tile([P, KB, T, MB, P], f16)
            wview = wbuf[:].rearrange("p m (i t) -> p m t i", t=T)  # strided view

            sp = wtmp.tile([P, KB, K], f32)
            spc = wtmp.tile([P, K], f32)
            junk = wtmp.tile([P, K * MB * P], f16)
            demod = const.tile([P, K], f32)
            dps = None  # psum slice for the cross-partition demod broadcast

            def emit_group(kw):
                # transpose the 6 (kh, mb) tiles of each kb into one psum tile,
                # then one style-scale op per kb, then the sum-of-squares
                for kb in range(KB):
                    pt = psum.tile([P, NU, CH_ROWS, NT], f32, name="pc")
                    ptf = pt[:].rearrange("p a b c -> p (a b c)")
                    ptv = ptf[:, 0 : K * MB * P].rearrange(
                        "p (kh mb o) -> p kh mb o", kh=K, mb=MB
                    )
                    # weight "transpose" via ident-matmul (w.T @ I = w^T) so
                    # the result lands in the f32 conv-psum tile directly
                    for kh in range(K):
                        t = kh * K + kw
                        for mb in range(MB):
                            nc.tensor.matmul(
                                ptv[:, kh, mb, :],
                                wview[:, mb, t, kb * P : (kb + 1) * P],
                                ident,
                                start=True, stop=True,
                            )
                    nc.scalar.activation(
                        out=wfin[:, kb, kw::K, :, :],
                        in_=ptv,
                        func=Act.Copy,
                        scale=stile[:, kb : kb + 1],
                    )
                # demod[kw] numerator: sum of squares over (o, i, kh) in ONE
                # ACT op per kb: Square(wfin) with free-dim accumulator
                for kb in range(KB):
                    nc.scalar.activation(
                        out=junk[:].rearrange("p (a b c) -> p a b c", a=K, b=MB),
                        in_=wfin[:, kb, kw::K, :, :],
                        func=Act.Square,
                        accum_out=sp[:, kb, kw : kw + 1],
                    )
                nc.vector.tensor_add(
                    out=spc[:, kw : kw + 1],
                    in0=sp[:, 0, kw : kw + 1],
                    in1=sp[:, 1, kw : kw + 1],
                )

            def emit_tail(kw):
                # cross-partition sum broadcast to all partitions, demod chain
                nc.tensor.matmul(
                    dps[:, kw : kw + 1], ones, spc[:, kw : kw + 1],
                    start=True, stop=True,
                )
                nc.vector.tensor_scalar_add(
                    demod[:, kw : kw + 1], dps[:, kw : kw + 1], 1e-8
                )
                nc.scalar.sqrt(demod[:, kw : kw + 1], demod[:, kw : kw + 1])
                nc.vector.reciprocal(demod[:, kw : kw + 1], demod[:, kw : kw + 1])

            # all groups first; the dps tile is then allocated LAST from the
            # rotating psum pool so no later wprep allocation reclaims its
            # buffer while the demod tails still read it
            emit_group(0)
            emit_group(1)
            emit_group(2)
            dpst = psum.tile([P, NU, CH_ROWS, NT], f32, name="pc")
            dps = dpst[:, 0, 0, 0:K]
            emit_tail(0)
            emit_tail(1)
            emit_tail(2)

            # ---------- input-transform helpers ----------
            helpers = {}

            def emit_helpers(p, kb):
                xv = xq[:, kb, p * PROWS : p * PROWS + PPAD, :]  # [P, PPAD, WP]
                A = ttmp.tile([P, PPAD, W], f16, name="A")
                TT = ttmp.tile([P, PPAD, W], f16, name="T")
                DD = ttmp.tile([P, PPAD, W], f16, name="D")
                nc.vector.tensor_sub(A, xv[:, :, 0:W], xv[:, :, 2 : W + 2])
                nc.vector.tensor_add(TT, xv[:, :, 1 : W + 1], xv[:, :, 2 : W + 2])
                nc.vector.tensor_sub(DD, xv[:, :, 1 : W + 1], xv[:, :, 2 : W + 2])
                helpers[(p, kb)] = (A, TT, DD)

            def rhs_view(p, kb, u, lr0, kh):
                A, TT, DD = helpers[(p, kb)]
                rows = slice(lr0 + kh, lr0 + kh + CH_ROWS)
                if u == 0:
                    return A[:, rows, 0::2]
                if u == 1:
                    return TT[:, rows, 0::2]
                if u == 2:
                    return DD[:, rows, 0::2]
                return A[:, rows, 1::2]

            # ---------- winograd coefficients + transformed weights ----------
            cu = const.tile([P, NU, K], f32)
            for u in range(NU):
                for kw in range(K):
                    g = G_ROWS[u][kw]
                    if g != 0.0:
                        nc.vector.tensor_scalar_mul(
                            out=cu[:, u, kw : kw + 1],
                            in0=demod[:, kw : kw + 1],
                            scalar1=float(USCALE * g * U_SIGN[u]),
                        )

            Ub = const.tile([P, KB, K, NU, MB, P], f16)

            def emit_U(u, mb):
                # 1-term rows on ACT (copy-scale); 3-term rows as DVE chains
                # (keeps the ACT and DVE streams both short at startup)
                nz = [kw for kw in range(K) if G_ROWS[u][kw] != 0.0]
                for kb in range(KB):
                    for kh in range(K):
                        srcs = [wfin[:, kb, kh * K + kw, mb, :] for kw in nz]
                        dst = Ub[:, kb, kh, u, mb, :]
                        if len(nz) == 1:
                            nc.scalar.activation(
                                out=dst, in_=srcs[0], func=Act.Copy,
                                scale=cu[:, u, nz[0] : nz[0] + 1],
                            )
                        else:
                            ta = utmp.tile([P, P], f32, name="ta")
                            nc.vector.tensor_scalar_mul(
                                out=ta, in0=srcs[0],
                                scalar1=cu[:, u, nz[0] : nz[0] + 1],
                            )
                            tb = utmp.tile([P, P], f32, name="tb")
                            nc.vector.scalar_tensor_tensor(
                                out=tb, in0=srcs[1],
                                scalar=cu[:, u, nz[1] : nz[1] + 1],
                                in1=ta, op0=Alu.mult, op1=Alu.add,
                            )
                            nc.vector.scalar_tensor_tensor(
                                out=dst, in0=srcs[2],
                                scalar=cu[:, u, nz[2] : nz[2] + 1],
                                in1=tb, op0=Alu.mult, op1=Alu.add,
                            )

            # piece-0 helpers can run on DVE while ACT/PE finish the demod;
            # U built mb-major so chunk0-mb0's weights are ready first
            emit_helpers(0, 0)
            emit_helpers(0, 1)
            for mb in range(MB):
                for u in range(NU):
                    emit_U(u, mb)

            if stage == "wprep":
                ot = outp.tile([P, KB * K * NU * MB * P], f16)
                nc.vector.tensor_copy(
                    out=ot, in_=Ub[:].rearrange("p a b c d e -> p (a b c d e)")
                )
                nc.sync.dma_start(out=y_flat[0:P, 0 : KB * K * NU * MB * P], in_=ot)
                ot2 = outp.tile([P, K], f16)
                nc.vector.tensor_copy(out=ot2, in_=demod)
                nc.sync.dma_start(out=y_flat[0:P, 16000 : 16000 + K], in_=ot2)

            if stage == "full":
                # ---------- conv over 8 pieces ----------
                def emit_chunk(p, c):
                    lr0 = c * CH_ROWS
                    r0 = p * PROWS + lr0
                    yt = outp.tile([P, MB, CH_ROWS, W], f32, name="yt")
                    ots = outp.tile([P, MB, CH_ROWS, W], f16, name="ot")
                    for mb in range(MB):
                        pt = psum.tile([P, NU, CH_ROWS, NT], f32, name="pc")
                        for u in range(NU):
                            first = True
                            for kb in range(KB):
                                for kh in range(K):
                                    nc.tensor.matmul(
                                        pt[:, u],
                                        Ub[:, kb, kh, u, mb, :],
                                        rhs_view(p, kb, u, lr0, kh),
                                        start=first,
                                        stop=(kb == KB - 1 and kh == K - 1),
                                    )
                                    first = False
                        # ---- epilogue ----
                        c1 = ctmp.tile([P, CH_ROWS, NT], f32, name="c1")
                        c2 = ctmp.tile([P, CH_ROWS, NT], f32, name="c2")
                        nc.scalar.activation(out=c1, in_=pt[:, 1], func=Act.Copy)
                        nc.scalar.activation(out=c2, in_=pt[:, 2], func=Act.Copy)
                        s01 = ctmp.tile([P, CH_ROWS, NT], f32, name="s01")
                        d12 = ctmp.tile([P, CH_ROWS, NT], f32, name="d12")
                        nc.vector.tensor_add(s01, c1, c2)
                        nc.vector.tensor_sub(d12, c1, c2)
                        ytv = yt[:, mb]
                        nc.vector.tensor_add(ytv[:, :, 0::2], s01, pt[:, 0])
                        nc.vector.tensor_sub(ytv[:, :, 1::2], d12, pt[:, 3])
                        # leaky relu + 1/64 descale + f16 cast on ACT
                        nc.scalar.activation(
                            out=ots[:, mb], in_=yt[:, mb], func=Act.Prelu,
                            scale=1.0 / USCALE, alpha=0.2)
                    nc.sync.dma_start(
                        out=y_pmf[:, :, r0 * W : r0 * W + CH_ROWS * W],
                        in_=ots,
                    )

                for p in range(NP):
                    emit_chunk(p, 0)
                    # both next-piece helpers NOW: kb1 must land before the
                    # next piece's first chunk or the PE gaps at the boundary
                    if p + 1 < NP:
                        emit_helpers(p + 1, 0)
                        emit_helpers(p + 1, 1)
                    emit_chunk(p, 1)
    nc.compile()
    return nc


def _get_nc():
    if "nc" not in _CACHE:
        _CACHE["nc"] = _build()
    return _CACHE["nc"]


def prep_in_maps(input_vector, style_vector, weight):
    """Host-side staging: fp16 casts, per-core input dicts."""
    x16 = np.ascontiguousarray(input_vector, dtype=np.float16)
    w16 = np.ascontiguousarray(weight, dtype=np.float16)
    s32 = np.ascontiguousarray(style_vector, dtype=np.float32)
    return [
        {"x": x16[b], "style": s32[b : b + 1], "w": w16}
        for b in range(B)
    ]


def _get_runner():
    """Build (once) a reusable jitted shard_map runner over the 8 cores, so
    repeated kernel() calls skip re-tracing/lowering the bass module."""
    if "runner" in _CACHE:
        return _CACHE["runner"]

    import jax
    import concourse.bass2jax as b2j
    import concourse.mybir as mybir
    from jax.experimental.shard_map import shard_map
    from jax.sharding import Mesh, PartitionSpec

    nc = _get_nc()
    b2j.install_neuronx_cc_hook()

    partition_name = nc.partition_id_tensor.name if nc.partition_id_tensor else None
    in_names, out_names, out_avals, zero_outs = [], [], [], []
    for alloc in nc.m.functions[0].allocations:
        if not isinstance(alloc, mybir.MemoryLocationSet):
            continue
        name = alloc.memorylocations[0].name
        if alloc.kind == "ExternalInput":
            if name != partition_name:
                in_names.append(name)
        elif alloc.kind == "ExternalOutput":
            out_names.append(name)
            shape = tuple(alloc.tensor_shape)
            dtype = mybir.dt.np(alloc.dtype)
            out_avals.append(jax.core.ShapedArray(shape, dtype))
            zero_outs.append(np.zeros(shape, dtype))
    n_params = len(in_names)
    n_outs = len(out_avals)
    all_in_names = list(in_names) + list(out_names)
    if partition_name is not None:
        all_in_names.append(partition_name)

    def _body(*args):
        operands = list(args)
        if partition_name is not None:
            operands.append(b2j.partition_id_tensor())
        outs = b2j._bass_exec_p.bind(
            *operands,
            out_avals=tuple(out_avals),
            in_names=tuple(all_in_names),
            out_names=tuple(out_names),
            lowering_input_output_aliases=(),
            sim_require_finite=True,
            sim_require_nnan=True,
            nc=nc,
        )
        return tuple(outs)

    devices = jax.devices()[:B]
    mesh = Mesh(np.asarray(devices), ("core",))
    in_specs = (PartitionSpec("core"),) * (n_params + n_outs)
    out_specs = (PartitionSpec("core"),) * len(out_names)
    sharded = jax.jit(
        shard_map(_body, mesh=mesh, in_specs=in_specs, out_specs=out_specs,
                  check_rep=False),
        donate_argnums=tuple(range(n_params, n_params + n_outs)),
        keep_unused=True,
    )
    _CACHE["runner"] = (sharded, in_names, out_names, out_avals, zero_outs)
    return _CACHE["runner"]


def finish_out(y_stack):
    """Raw stacked per-core outputs (B, COUT, H, W) -> full f32 output."""
    return np.ascontiguousarray(y_stack).astype(np.float32)


def kernel(input_vector, style_vector, weight):
    in_maps = prep_in_maps(input_vector, style_vector, weight)
    try:
        sharded, in_names, out_names, out_avals, zero_outs = _get_runner()
        concat_in = [
            np.concatenate([in_maps[c][nm] for c in range(B)], axis=0)
            for nm in in_names
        ]
        zeros = [
            np.zeros((B * z.shape[0], *z.shape[1:]), z.dtype) for z in zero_outs
        ]
        out_arrs = sharded(*concat_in, *zeros)
        yi = out_names.index("y")
        out = np.asarray(out_arrs[yi]).reshape(B, *out_avals[yi].shape)
    except Exception:
        # fallback: the one-shot path (slower per call, same result)
        from concourse.bass_utils import run_bass_kernel_spmd

        _CACHE.pop("runner", None)
        res = run_bass_kernel_spmd(_get_nc(), in_maps, core_ids=list(range(B)))
        out = np.stack([res.results[b]["y"] for b in range(B)], axis=0)
    return out.astype(np.float32)


# revision 21
# speedup vs baseline: 1.1758x; 1.1758x over previous
"""Trainium2 Bass kernel for per-sample weight-demodulated 3x3 conv + leaky ReLU.

Problem (hardcoded shapes):
  input_vector: (8, 256, 128, 128) f32
  style_vector: (8, 256) f32
  weight:       (256, 256, 3, 3) f32
  out:          (8, 256, 128, 128) f32

Math (faithful to reference):
  ws[b,o,i,kh,kw] = weight[o,i,kh,kw] * style[b,i]
  demod[b,kw]     = rsqrt(sum_{o,i,kh} ws^2 + 1e-8)        # NOTE: sum excludes kw
  y[b] = leaky_relu(conv2d_same(x[b], ws[b]*demod), 0.2)

Sharding: data-parallel over batch, one sample per NeuronCore (8 cores).

Per-core kernel: 1D Winograd F(2,3) along the kw axis (kh stays direct),
cutting PE matmul work 1.5x vs the direct conv while keeping the PE the
dominant engine (the HAM clock manager holds full clock only under a
sustained-busy engine; a balanced multi-engine design oscillates at k=4).

  - weight prep: w DMA'd f16, PE-transposed, ACT style-scaled into wfin
    f16, demod per kw via ACT Square+accum and a ones-matmul broadcast.
  - the F(2,3) kernel transform G = [[1,0,0],[.5,.5,.5],[.5,-.5,.5],
    [0,0,1]] folds with demod into per-partition coefficients
    cu[u,kw] = 64*G*demod (u=2 negated, see below); U[u,kh] built by ACT
    copy-scale + DVE scalar_tensor_tensor chains, f16.  The x64 scale
    keeps U out of f16-denormal range; compensated exactly by scale=1/64
    inside the final ACT Prelu.
  - input transform: the F(2,3) data components are EXACTLY strided views
    of three contiguous helper tensors per (piece, cin-half):
        A_c = x_c - x_{c+2}   -> d0 = A[0::2],  d3 = A[1::2]
        T_c = x_{c+1}+x_{c+2} -> d1 = T[0::2]
        D_c = x_{c+1}-x_{c+2} -> d2 = -D[0::2]  (sign folded into cu[2])
    so DVE does just 3 packed-f16 tensor ops per (piece, cin-half) and the
    PE matmuls read the strided views directly.
  - conv: per 8-row chunk and cout-half, 24 f16 matmuls (4 u-comps x 3 kh
    x 2 cin-halves, free = 8 rows x 64 tiles = 512) accumulate into one
    [128,4,8,64] f32 PSUM tile (4 banks, double-buffered = all 8; the PE
    warmup ramp and the demod broadcast borrow tiles from the same pool).
  - epilogue: ACT copies m1,m2 from PSUM; DVE: y[2t] = (m1+m2)+m0,
    y[2t+1] = (m1-m2)-m3 (4 ops); ACT applies leaky-ReLU natively
    (parametric_relu alpha=0.2, scale=1/64, same ACT function set as
    Copy/Square/Sqrt - no table swaps) casting f16; one merged y DMA per
    chunk on the SP queue.
  - x lives in ONE big padded SBUF tile written once by disjoint per-piece
    sub-DMAs (no buffer reuse -> no DMA WAR races; overlap preserved).
"""

import numpy as np

B, CIN, COUT, K, H, W = 8, 256, 256, 3, 128, 128
P = 128
KB = CIN // P   # cin partition blocks   = 2
MB = COUT // P  # cout partition blocks  = 2
T = K * K       # taps = 9
NU = 4          # winograd F(2,3) components
NT = W // 2     # winograd tiles per row = 64
WP = W + 2      # padded row width = 130
NP = 8          # x pieces
PROWS = H // NP           # output rows per piece = 16
PPAD = PROWS + 2          # padded rows per piece = 18
CH_ROWS = 8               # output rows per psum chunk
CHUNKS = PROWS // CH_ROWS  # = 2
N_WARMUP = 40
USCALE = 64.0

# F(2,3) kernel transform G, rows u=0..3 over kw=0..2; u=2 sign-flipped
# because its data component is read as D = -(d2 view).
G_ROWS = [
    [1.0, 0.0, 0.0],
    [0.5, 0.5, 0.5],
    [0.5, -0.5, 0.5],
    [0.0, 0.0, 1.0],
]
U_SIGN = [1.0, 1.0, -1.0, 1.0]

_CACHE = {}


def _build(stage="full"):
    import concourse.mybir as mybir
    import concourse.tile as tile
    from concourse import bacc
    from concourse.masks import make_identity

    f32 = mybir.dt.float32
    f16 = mybir.dt.float16
    Alu = mybir.AluOpType
    Act = mybir.ActivationFunctionType

    nc = bacc.Bacc(None, target_bir_lowering=False)
    x_d = nc.dram_tensor("x", [CIN, H, W], f16, kind="ExternalInput")
    s_d = nc.dram_tensor("style", [1, CIN], f32, kind="ExternalInput")
    w_d = nc.dram_tensor("w", [COUT, CIN, K, K], f16, kind="ExternalInput")
    y_d = nc.dram_tensor("y", [COUT, H, W], f16, kind="ExternalOutput")

    y_flat = y_d[:].rearrange("o h w -> o (h w)")      # [256, 16384]
    y_pmf = y_d[:].rearrange("(m p) h w -> p m (h w)", p=P)  # [128, 2, 16384]
    w_flat = w_d[:].rearrange("o i kh kw -> o (i kh kw)")  # [256, 2304]

    with tile.TileContext(nc) as tc:
        with (
            tc.tile_pool(name="const", bufs=1) as const,
            tc.tile_pool(name="wtmp", bufs=1) as wtmp,
            tc.tile_pool(name="utmp", bufs=2) as utmp,
            tc.tile_pool(name="xbuf", bufs=1) as xbuf,
            tc.tile_pool(name="ttmp", bufs=4) as ttmp,
            tc.tile_pool(name="ctmp", bufs=2) as ctmp,
            tc.tile_pool(name="outp", bufs=2) as outp,
            tc.tile_pool(name="psum", bufs=2, space="PSUM") as psum,
        ):
            # ---------- constants ----------
            ident = const.tile([P, P], f16)
            make_identity(nc, ident)
            ones = const.tile([P, P], f32)
            nc.vector.memset(ones, 1.0)
            # dummy sqrt so the ACT function-set containing Sqrt (and Copy/
            # Square/parametric_relu) loads now, not mid-prep
            nc.scalar.sqrt(ones[0:1, 0:1], ones[0:1, 0:1])

            # ---------- weight load: very first DMA on the SP queue ----------
            wbuf = wtmp.tile([P, MB, CIN * T], f16)
            nc.sync.dma_start(
                out=wbuf[:],
                in_=w_flat.rearrange("(m p) f -> p m f", p=P),
            )

            # style per-partition: stile[p, kb] = style[kb*128 + p]
            stile = const.tile([P, KB], f32)
            for kb in range(KB):
                nc.sync.dma_start(
                    out=stile[:, kb : kb + 1],
                    in_=s_d[:].rearrange("one c -> c one")[kb * P : (kb + 1) * P, :],
                )

            # ---------- x: one big padded tile, disjoint piece sub-DMAs ------
            xq = xbuf.tile([P, KB, H + 2, WP], f16, name="xq")
            for kb in range(KB):
                nc.gpsimd.memset(xq[:, kb, :, 0], 0.0)
                nc.gpsimd.memset(xq[:, kb, :, WP - 1], 0.0)
                nc.gpsimd.memset(xq[:, kb, 0, :], 0.0)
                nc.gpsimd.memset(xq[:, kb, H + 1, :], 0.0)
            for p in range(NP):
                r_lo = p * PROWS
                for kb in range(KB):
                    nc.sync.dma_start(
                        out=xq[:, kb, r_lo + 1 : r_lo + 1 + PROWS, 1 : 1 + W],
                        in_=x_d[kb * P : (kb + 1) * P, r_lo : r_lo + PROWS, :],
                    )

            # ---------- PE warmup: ramp the clock while the w DMA flies ------
            # (borrows conv-psum tiles: matmul of ident into the f32 tile)
            for _ in range(N_WARMUP):
                gate = psum.tile([P, NU, CH_ROWS, NT], f32, name="pc")
                nc.tensor.matmul(
                    gate[:, 0, 0:2, :], ident, ident, start=True, stop=True
                )

            # ---------- weight prep, kw-major ----------
            wfin = const.tile([P, KB, T, MB, P], f16)
            wview = wbuf[:].rearrange("p m (i t) -> p m t i", t=T)  # strided view

            sp = wtmp.tile([P, KB, K], f32)
            spc = wtmp.tile([P, K], f32)
            junk = wtmp.tile([P, K * MB * P], f16)
            demod = const.tile([P, K], f32)
            dps = None  # psum slice for the cross-partition demod broadcast

            def emit_group(kw):
                # transpose the 6 (kh, mb) tiles of each kb into one psum tile,
                # then one style-scale op per kb, then the sum-of-squares
                for kb in range(KB):
                    pt = psum.tile([P, NU, CH_ROWS, NT], f32, name="pc")
                    ptf = pt[:].rearrange("p a b c -> p (a b c)")
                    ptv = ptf[:, 0 : K * MB * P].rearrange(
                        "p (kh mb o) -> p kh mb o", kh=K, mb=MB
                    )
                    # weight "transpose" via ident-matmul (w.T @ I = w^T) so
                    # the result lands in the f32 conv-psum tile directly
                    for kh in range(K):
                        t = kh * K + kw
                        for mb in range(MB):
                            nc.tensor.matmul(
                                ptv[:, kh, mb, :],
                                wview[:, mb, t, kb * P : (kb + 1) * P],
                                ident,
                                start=True, stop=True,
                            )
                    nc.scalar.activation(
                        out=wfin[:, kb, kw::K, :, :],
                        in_=ptv,
                        func=Act.Copy,
                        scale=stile[:, kb : kb + 1],
                    )
                # demod[kw] numerator: sum of squares over (o, i, kh) in ONE
                # ACT op per kb: Square(wfin) with free-dim accumulator
                for kb in range(KB):
                    nc.scalar.activation(
                        out=junk[:].rearrange("p (a b c) -> p a b c", a=K, b=MB),
                        in_=wfin[:, kb, kw::K, :, :],
                        func=Act.Square,
                        accum_out=sp[:, kb, kw : kw + 1],
                    )
                nc.vector.tensor_add(
                    out=spc[:, kw : kw + 1],
                    in0=sp[:, 0, kw : kw + 1],
                    in1=sp[:, 1, kw : kw + 1],
                )

            def emit_tail(kw):
                # cross-partition sum broadcast to all partitions, demod chain
                nc.tensor.matmul(
                    dps[:, kw : kw + 1], ones, spc[:, kw : kw + 1],
                    start=True, stop=True,
                )
                nc.vector.tensor_scalar_add(
                    demod[:, kw : kw + 1], dps[:, kw : kw + 1], 1e-8
                )
                nc.scalar.sqrt(demod[:, kw : kw + 1], demod[:, kw : kw + 1])
                nc.vector.reciprocal(demod[:, kw : kw + 1], demod[:, kw : kw + 1])

            # all groups first; the dps tile is then allocated LAST from the
            # rotating psum pool so no later wprep allocation reclaims its
            # buffer while the demod tails still read it
            emit_group(0)
            emit_group(1)
            emit_group(2)
            dpst = psum.tile([P, NU, CH_ROWS, NT], f32, name="pc")
            dps = dpst[:, 0, 0, 0:K]
            emit_tail(0)
            emit_tail(1)
            emit_tail(2)

            # ---------- input-transform helpers ----------
            helpers = {}

            def emit_helpers(p, kb):
                xv = xq[:, kb, p * PROWS : p * PROWS + PPAD, :]  # [P, PPAD, WP]
                A = ttmp.tile([P, PPAD, W], f16, name="A")
                TT = ttmp.tile([P, PPAD, W], f16, name="T")
                DD = ttmp.tile([P, PPAD, W], f16, name="D")
                nc.vector.tensor_sub(A, xv[:, :, 0:W], xv[:, :, 2 : W + 2])
                nc.vector.tensor_add(TT, xv[:, :, 1 : W + 1], xv[:, :, 2 : W + 2])
                nc.vector.tensor_sub(DD, xv[:, :, 1 : W + 1], xv[:, :, 2 : W + 2])
                helpers[(p, kb)] = (A, TT, DD)

            def rhs_view(p, kb, u, lr0, kh):
                A, TT, DD = helpers[(p, kb)]
                rows = slice(lr0 + kh, lr0 + kh + CH_ROWS)
                if u == 0:
                    return A[:, rows, 0::2]
                if u == 1:
                    return TT[:, rows, 0::2]
                if u == 2:
                    return DD[:, rows, 0::2]
                return A[:, rows, 1::2]

            # ---------- winograd coefficients + transformed weights ----------
            cu = const.tile([P, NU, K], f32)
            for u in range(NU):
                for kw in range(K):
                    g = G_ROWS[u][kw]
                    if g != 0.0:
                        nc.vector.tensor_scalar_mul(
                            out=cu[:, u, kw : kw + 1],
                            in0=demod[:, kw : kw + 1],
                            scalar1=float(USCALE * g * U_SIGN[u]),
                        )

            Ub = const.tile([P, KB, K, NU, MB, P], f16)

            def emit_U(u, mb):
                # 1-term rows on ACT (copy-scale); 3-term rows as DVE chains
                # (keeps the ACT and DVE streams both short at startup)
                nz = [kw for kw in range(K) if G_ROWS[u][kw] != 0.0]
                for kb in range(KB):
                    for kh in range(K):
                        srcs = [wfin[:, kb, kh * K + kw, mb, :] for kw in nz]
                        dst = Ub[:, kb, kh, u, mb, :]
                        if len(nz) == 1:
                            nc.scalar.activation(
                                out=dst, in_=srcs[0], func=Act.Copy,
                                scale=cu[:, u, nz[0] : nz[0] + 1],
                            )
                        else:
                            ta = utmp.tile([P, P], f32, name="ta")
                            nc.vector.tensor_scalar_mul(
                                out=ta, in0=srcs[0],
                                scalar1=cu[:, u, nz[0] : nz[0] + 1],
                            )
                            tb = utmp.tile([P, P], f32, name="tb")
                            nc.vector.scalar_tensor_tensor(
                                out=tb, in0=srcs[1],
                                scalar=cu[:, u, nz[1] : nz[1] + 1],
                                in1=ta, op0=Alu.mult, op1=Alu.add,
                            )
                            nc.vector.scalar_tensor_tensor(
                                out=dst, in0=srcs[2],
                                scalar=cu[:, u, nz[2] : nz[2] + 1],
                                in1=tb, op0=Alu.mult, op1=Alu.add,
                            )

            # piece-0 helpers first: conv u0 needs them with U[u0]
            emit_helpers(0, 0)
            emit_helpers(0, 1)
            # U built mb-major so chunk0-mb0's weights are ready first
            for mb in range(MB):
                for u in range(NU):
                    emit_U(u, mb)

            if stage == "wprep":
                ot = outp.tile([P, KB * K * NU * MB * P], f16)
                nc.vector.tensor_copy(
                    out=ot, in_=Ub[:].rearrange("p a b c d e -> p (a b c d e)")
                )
                nc.sync.dma_start(out=y_flat[0:P, 0 : KB * K * NU * MB * P], in_=ot)
                ot2 = outp.tile([P, K], f16)
                nc.vector.tensor_copy(out=ot2, in_=demod)
                nc.sync.dma_start(out=y_flat[0:P, 16000 : 16000 + K], in_=ot2)

            if stage == "full":
                # ---------- conv over 8 pieces ----------
                def emit_chunk(p, c):
                    lr0 = c * CH_ROWS
                    r0 = p * PROWS + lr0
                    ots = outp.tile([P, MB, CH_ROWS, W], f16, name="ot")
                    for mb in range(MB):
                        yt = outp.tile([P, CH_ROWS, W], f32, name="yt")
                        pt = psum.tile([P, NU, CH_ROWS, NT], f32, name="pc")
                        for u in range(NU):
                            first = True
                            for kb in range(KB):
                                for kh in range(K):
                                    nc.tensor.matmul(
                                        pt[:, u],
                                        Ub[:, kb, kh, u, mb, :],
                                        rhs_view(p, kb, u, lr0, kh),
                                        start=first,
                                        stop=(kb == KB - 1 and kh == K - 1),
                                    )
                                    first = False
                        # ---- epilogue ----
                        c1 = ctmp.tile([P, CH_ROWS, NT], f32, name="c1")
                        c2 = ctmp.tile([P, CH_ROWS, NT], f32, name="c2")
                        nc.scalar.activation(out=c1, in_=pt[:, 1], func=Act.Copy)
                        nc.scalar.activation(out=c2, in_=pt[:, 2], func=Act.Copy)
                        s01 = ctmp.tile([P, CH_ROWS, NT], f32, name="s01")
                        d12 = ctmp.tile([P, CH_ROWS, NT], f32, name="d12")
                        nc.vector.tensor_add(s01, c1, c2)
                        nc.vector.tensor_sub(d12, c1, c2)
                        ytv = yt
                        nc.vector.tensor_add(ytv[:, :, 0::2], s01, pt[:, 0])
                        nc.vector.tensor_sub(ytv[:, :, 1::2], d12, pt[:, 3])
                        # leaky relu + 1/64 descale + f16 cast on ACT
                        nc.scalar.activation(
                            out=ots[:, mb], in_=yt, func=Act.Prelu,
                            scale=1.0 / USCALE, alpha=0.2)
                    nc.sync.dma_start(
                        out=y_pmf[:, :, r0 * W : r0 * W + CH_ROWS * W],
                        in_=ots,
                    )

                for p in range(NP):
                    # next-piece helpers FIRST in the DVE queue: they depend
                    # only on the x DMA, so they fill DVE idle time while the
                    # PE chews piece p, and are long done at the boundary
                    if p + 1 < NP:
                        emit_helpers(p + 1, 0)
                        emit_helpers(p + 1, 1)
                    emit_chunk(p, 0)
                    emit_chunk(p, 1)
    nc.compile()
    return nc


def _get_nc():
    if "nc" not in _CACHE:
        _CACHE["nc"] = _build()
    return _CACHE["nc"]


def prep_in_maps(input_vector, style_vector, weight):
    """Host-side staging: fp16 casts, per-core input dicts."""
    x16 = np.ascontiguousarray(input_vector, dtype=np.float16)
    w16 = np.ascontiguousarray(weight, dtype=np.float16)
    s32 = np.ascontiguousarray(style_vector, dtype=np.float32)
    return [
        {"x": x16[b], "style": s32[b : b + 1], "w": w16}
        for b in range(B)
    ]


def _get_runner():
    """Build (once) a reusable jitted shard_map runner over the 8 cores, so
    repeated kernel() calls skip re-tracing/lowering the bass module."""
    if "runner" in _CACHE:
        return _CACHE["runner"]

    import jax
    import concourse.bass2jax as b2j
    import concourse.mybir as mybir
    from jax.experimental.shard_map import shard_map
    from jax.sharding import Mesh, PartitionSpec

    nc = _get_nc()
    b2j.install_neuronx_cc_hook()

    partition_name = nc.partition_id_tensor.name if nc.partition_id_tensor else None
    in_names, out_names, out_avals, zero_outs = [], [], [], []
    for alloc in nc.m.functions[0].allocations:
        if not isinstance(alloc, mybir.MemoryLocationSet):
            continue
        name = alloc.memorylocations[0].name
        if alloc.kind == "ExternalInput":
            if name != partition_name:
                in_names.append(name)
        elif alloc.kind == "ExternalOutput":
            out_names.append(name)
            shape = tuple(alloc.tensor_shape)
            dtype = mybir.dt.np(alloc.dtype)
            out_avals.append(jax.core.ShapedArray(shape, dtype))
            zero_outs.append(np.zeros(shape, dtype))
    n_params = len(in_names)
    n_outs = len(out_avals)
    all_in_names = list(in_names) + list(out_names)
    if partition_name is not None:
        all_in_names.append(partition_name)

    def _body(*args):
        operands = list(args)
        if partition_name is not None:
            operands.append(b2j.partition_id_tensor())
        outs = b2j._bass_exec_p.bind(
            *operands,
            out_avals=tuple(out_avals),
            in_names=tuple(all_in_names),
            out_names=tuple(out_names),
            lowering_input_output_aliases=(),
            sim_require_finite=True,
            sim_require_nnan=True,
            nc=nc,
        )
        return tuple(outs)

    devices = jax.devices()[:B]
    mesh = Mesh(np.asarray(devices), ("core",))
    in_specs = (PartitionSpec("core"),) * (n_params + n_outs)
    out_specs = (PartitionSpec("core"),) * len(out_names)
    sharded = jax.jit(
        shard_map(_body, mesh=mesh, in_specs=in_specs, out_specs=out_specs,
                  check_rep=False),
        donate_argnums=tuple(range(n_params, n_params + n_outs)),
        keep_unused=True,
    )
    _CACHE["runner"] = (sharded, in_names, out_names, out_avals, zero_outs)
    return _CACHE["runner"]


def finish_out(y_stack):
    """Raw stacked per-core outputs (B, COUT, H, W) -> full f32 output."""
    return np.ascontiguousarray(y_stack).astype(np.float32)


def kernel(input_vector, style_vector, weight):
    in_maps = prep_in_maps(input_vector, style_vector, weight)
    try:
        sharded, in_names, out_names, out_avals, zero_outs = _get_runner()
        concat_in = [
            np.concatenate([in_maps[c][nm] for c in range(B)], axis=0)
            for nm in in_names
        ]
        zeros = [
            np.zeros((B * z.shape[0], *z.shape[1:]), z.dtype) for z in zero_outs
        ]
        out_arrs = sharded(*concat_in, *zeros)
        yi = out_names.index("y")
        out = np.asarray(out_arrs[yi]).reshape(B, *out_avals[yi].shape)
    except Exception:
        # fallback: the one-shot path (slower per call, same result)
        from concourse.bass_utils import run_bass_kernel_spmd

        _CACHE.pop("runner", None)
        res = run_bass_kernel_spmd(_get_nc(), in_maps, core_ids=list(range(B)))
        out = np.stack([res.results[b]["y"] for b in range(B)], axis=0)
    return out.astype(np.float32)


# revision 22
# speedup vs baseline: 1.2098x; 1.0289x over previous
"""Trainium2 Bass kernel for per-sample weight-demodulated 3x3 conv + leaky ReLU.

Problem (hardcoded shapes):
  input_vector: (8, 256, 128, 128) f32
  style_vector: (8, 256) f32
  weight:       (256, 256, 3, 3) f32
  out:          (8, 256, 128, 128) f32

Math (faithful to reference):
  ws[b,o,i,kh,kw] = weight[o,i,kh,kw] * style[b,i]
  demod[b,kw]     = rsqrt(sum_{o,i,kh} ws^2 + 1e-8)        # NOTE: sum excludes kw
  y[b] = leaky_relu(conv2d_same(x[b], ws[b]*demod), 0.2)

Sharding: data-parallel over batch, one sample per NeuronCore (8 cores).

Per-core kernel: 1D Winograd F(2,3) along the kw axis (kh stays direct),
cutting PE matmul work 1.5x vs the direct conv while keeping the PE the
dominant engine (the HAM clock manager holds full clock only under a
sustained-busy engine; a balanced multi-engine design oscillates at k=4).

  - weight prep: w DMA'd f16, PE-transposed, ACT style-scaled into wfin
    f16, demod per kw via ACT Square+accum and a ones-matmul broadcast.
  - the F(2,3) kernel transform G = [[1,0,0],[.5,.5,.5],[.5,-.5,.5],
    [0,0,1]] folds with demod into per-partition coefficients
    cu[u,kw] = 64*G*demod (u=2 negated, see below); U[u,kh] built by ACT
    copy-scale + DVE scalar_tensor_tensor chains, f16.  The x64 scale
    keeps U out of f16-denormal range; compensated exactly by scale=1/64
    inside the final ACT Prelu.
  - input transform: the F(2,3) data components are EXACTLY strided views
    of three contiguous helper tensors per (piece, cin-half):
        A_c = x_c - x_{c+2}   -> d0 = A[0::2],  d3 = A[1::2]
        T_c = x_{c+1}+x_{c+2} -> d1 = T[0::2]
        D_c = x_{c+1}-x_{c+2} -> d2 = -D[0::2]  (sign folded into cu[2])
    so DVE does just 3 packed-f16 tensor ops per (piece, cin-half) and the
    PE matmuls read the strided views directly.
  - conv: per 8-row chunk and cout-half, 24 f16 matmuls (4 u-comps x 3 kh
    x 2 cin-halves, free = 8 rows x 64 tiles = 512) accumulate into one
    [128,4,8,64] f32 PSUM tile (4 banks, double-buffered = all 8; the PE
    warmup ramp and the demod broadcast borrow tiles from the same pool).
  - epilogue: ACT copies m1,m2 from PSUM; DVE: y[2t] = (m1+m2)+m0,
    y[2t+1] = (m1-m2)-m3 (4 ops); ACT applies leaky-ReLU natively
    (parametric_relu alpha=0.2, scale=1/64, same ACT function set as
    Copy/Square/Sqrt - no table swaps) casting f16; one merged y DMA per
    chunk on the SP queue.
  - x lives in ONE big padded SBUF tile written once by disjoint per-piece
    sub-DMAs (no buffer reuse -> no DMA WAR races; overlap preserved).
"""

import numpy as np

B, CIN, COUT, K, H, W = 8, 256, 256, 3, 128, 128
P = 128
KB = CIN // P   # cin partition blocks   = 2
MB = COUT // P  # cout partition blocks  = 2
T = K * K       # taps = 9
NU = 4          # winograd F(2,3) components
NT = W // 2     # winograd tiles per row = 64
WP = W + 2      # padded row width = 130
NP = 8          # x pieces
PROWS = H // NP           # output rows per piece = 16
PPAD = PROWS + 2          # padded rows per piece = 18
CH_ROWS = 8               # output rows per psum chunk
CHUNKS = PROWS // CH_ROWS  # = 2
N_WARMUP = 40
USCALE = 64.0

# F(2,3) kernel transform G, rows u=0..3 over kw=0..2; u=2 sign-flipped
# because its data component is read as D = -(d2 view).
G_ROWS = [
    [1.0, 0.0, 0.0],
    [0.5, 0.5, 0.5],
    [0.5, -0.5, 0.5],
    [0.0, 0.0, 1.0],
]
U_SIGN = [1.0, 1.0, -1.0, 1.0]

_CACHE = {}


def _build(stage="full"):
    import concourse.mybir as mybir
    import concourse.tile as tile
    from concourse import bacc
    from concourse.masks import make_identity

    f32 = mybir.dt.float32
    f16 = mybir.dt.float16
    Alu = mybir.AluOpType
    Act = mybir.ActivationFunctionType

    nc = bacc.Bacc(None, target_bir_lowering=False)
    x_d = nc.dram_tensor("x", [CIN, H, W], f16, kind="ExternalInput")
    s_d = nc.dram_tensor("style", [1, CIN], f32, kind="ExternalInput")
    w_d = nc.dram_tensor("w", [COUT, CIN, K, K], f16, kind="ExternalInput")
    y_d = nc.dram_tensor("y", [COUT, H, W], f16, kind="ExternalOutput")

    y_flat = y_d[:].rearrange("o h w -> o (h w)")      # [256, 16384]
    y_pmf = y_d[:].rearrange("(m p) h w -> p m (h w)", p=P)  # [128, 2, 16384]
    w_flat = w_d[:].rearrange("o i kh kw -> o (i kh kw)")  # [256, 2304]

    with tile.TileContext(nc) as tc:
        with (
            tc.tile_pool(name="const", bufs=1) as const,
            tc.tile_pool(name="wtmp", bufs=1) as wtmp,
            tc.tile_pool(name="utmp", bufs=2) as utmp,
            tc.tile_pool(name="xbuf", bufs=1) as xbuf,
            tc.tile_pool(name="ttmp", bufs=4) as ttmp,
            tc.tile_pool(name="ctmp", bufs=2) as ctmp,
            tc.tile_pool(name="outp", bufs=2) as outp,
            tc.tile_pool(name="psum", bufs=2, space="PSUM") as psum,
        ):
            # ---------- constants ----------
            ident = const.tile([P, P], f16)
            make_identity(nc, ident)
            ones = const.tile([P, P], f32)
            nc.vector.memset(ones, 1.0)
            # dummy sqrt so the ACT function-set containing Sqrt (and Copy/
            # Square/parametric_relu) loads now, not mid-prep
            nc.scalar.sqrt(ones[0:1, 0:1], ones[0:1, 0:1])

            # ---------- weight load: very first DMA on the SP queue ----------
            wbuf = wtmp.tile([P, MB, CIN * T], f16)
            nc.sync.dma_start(
                out=wbuf[:],
                in_=w_flat.rearrange("(m p) f -> p m f", p=P),
            )

            # style per-partition: stile[p, kb] = style[kb*128 + p]
            stile = const.tile([P, KB], f32)
            for kb in range(KB):
                nc.sync.dma_start(
                    out=stile[:, kb : kb + 1],
                    in_=s_d[:].rearrange("one c -> c one")[kb * P : (kb + 1) * P, :],
                )

            # ---------- x: one big padded tile, disjoint piece sub-DMAs ------
            xq = xbuf.tile([P, KB, H + 2, WP], f16, name="xq")
            for kb in range(KB):
                nc.gpsimd.memset(xq[:, kb, :, 0], 0.0)
                nc.gpsimd.memset(xq[:, kb, :, WP - 1], 0.0)
                nc.gpsimd.memset(xq[:, kb, 0, :], 0.0)
                nc.gpsimd.memset(xq[:, kb, H + 1, :], 0.0)
            for p in range(NP):
                r_lo = p * PROWS
                for kb in range(KB):
                    nc.sync.dma_start(
                        out=xq[:, kb, r_lo + 1 : r_lo + 1 + PROWS, 1 : 1 + W],
                        in_=x_d[kb * P : (kb + 1) * P, r_lo : r_lo + PROWS, :],
                    )

            # ---------- PE warmup: ramp the clock while the w DMA flies ------
            # (borrows conv-psum tiles: matmul of ident into the f32 tile)
            for _ in range(N_WARMUP):
                gate = psum.tile([P, NU, CH_ROWS, NT], f32, name="pc")
                nc.tensor.matmul(
                    gate[:, 0, 0:2, :], ident, ident, start=True, stop=True
                )

            # ---------- weight prep, kw-major ----------
            wfin = const.tile([P, KB, T, MB, P], f16)
            wview = wbuf[:].rearrange("p m (i t) -> p m t i", t=T)  # strided view

            sp = wtmp.tile([P, KB, K], f32)
            spc = wtmp.tile([P, K], f32)
            junk = wtmp.tile([P, K * MB * P], f16)
            demod = const.tile([P, K], f32)
            dps = None  # psum slice for the cross-partition demod broadcast

            def emit_group(kw):
                # transpose the 6 (kh, mb) tiles of each kb into one psum tile,
                # then one style-scale op per kb, then the sum-of-squares
                for kb in range(KB):
                    pt = psum.tile([P, NU, CH_ROWS, NT], f32, name="pc")
                    ptf = pt[:].rearrange("p a b c -> p (a b c)")
                    ptv = ptf[:, 0 : K * MB * P].rearrange(
                        "p (kh mb o) -> p kh mb o", kh=K, mb=MB
                    )
                    # weight "transpose" via ident-matmul (w.T @ I = w^T) so
                    # the result lands in the f32 conv-psum tile directly
                    for kh in range(K):
                        t = kh * K + kw
                        for mb in range(MB):
                            nc.tensor.matmul(
                                ptv[:, kh, mb, :],
                                wview[:, mb, t, kb * P : (kb + 1) * P],
                                ident,
                                start=True, stop=True,
                            )
                    nc.scalar.activation(
                        out=wfin[:, kb, kw::K, :, :],
                        in_=ptv,
                        func=Act.Copy,
                        scale=stile[:, kb : kb + 1],
                    )
                # demod[kw] numerator: sum of squares over (o, i, kh) in ONE
                # ACT op per kb: Square(wfin) with free-dim accumulator
                for kb in range(KB):
                    nc.scalar.activation(
                        out=junk[:].rearrange("p (a b c) -> p a b c", a=K, b=MB),
                        in_=wfin[:, kb, kw::K, :, :],
                        func=Act.Square,
                        accum_out=sp[:, kb, kw : kw + 1],
                    )
                nc.vector.tensor_add(
                    out=spc[:, kw : kw + 1],
                    in0=sp[:, 0, kw : kw + 1],
                    in1=sp[:, 1, kw : kw + 1],
                )

            def emit_tail(kw):
                # cross-partition sum broadcast to all partitions, demod chain
                nc.tensor.matmul(
                    dps[:, kw : kw + 1], ones, spc[:, kw : kw + 1],
                    start=True, stop=True,
                )
                nc.vector.tensor_scalar_add(
                    demod[:, kw : kw + 1], dps[:, kw : kw + 1], 1e-8
                )
                nc.scalar.sqrt(demod[:, kw : kw + 1], demod[:, kw : kw + 1])
                nc.vector.reciprocal(demod[:, kw : kw + 1], demod[:, kw : kw + 1])

            # all groups first; the dps tile is then allocated LAST from the
            # rotating psum pool so no later wprep allocation reclaims its
            # buffer while the demod tails still read it
            emit_group(0)
            emit_group(1)
            emit_group(2)
            dpst = psum.tile([P, NU, CH_ROWS, NT], f32, name="pc")
            dps = dpst[:, 0, 0, 0:K]
            emit_tail(0)
            emit_tail(1)
            emit_tail(2)

            # ---------- input-transform helpers ----------
            helpers = {}

            def emit_helpers(p, kb):
                xv = xq[:, kb, p * PROWS : p * PROWS + PPAD, :]  # [P, PPAD, WP]
                A = ttmp.tile([P, PPAD, W], f16, name="A")
                TT = ttmp.tile([P, PPAD, W], f16, name="T")
                DD = ttmp.tile([P, PPAD, W], f16, name="D")
                nc.vector.tensor_sub(A, xv[:, :, 0:W], xv[:, :, 2 : W + 2])
                nc.vector.tensor_add(TT, xv[:, :, 1 : W + 1], xv[:, :, 2 : W + 2])
                nc.vector.tensor_sub(DD, xv[:, :, 1 : W + 1], xv[:, :, 2 : W + 2])
                helpers[(p, kb)] = (A, TT, DD)

            def rhs_view(p, kb, u, lr0, kh, nrows=CH_ROWS):
                A, TT, DD = helpers[(p, kb)]
                rows = slice(lr0 + kh, lr0 + kh + nrows)
                if u == 0:
                    return A[:, rows, 0::2]
                if u == 1:
                    return TT[:, rows, 0::2]
                if u == 2:
                    return DD[:, rows, 0::2]
                return A[:, rows, 1::2]

            # ---------- winograd coefficients + transformed weights ----------
            cu = const.tile([P, NU, K], f32)
            for u in range(NU):
                for kw in range(K):
                    g = G_ROWS[u][kw]
                    if g != 0.0:
                        nc.vector.tensor_scalar_mul(
                            out=cu[:, u, kw : kw + 1],
                            in0=demod[:, kw : kw + 1],
                            scalar1=float(USCALE * g * U_SIGN[u]),
                        )

            Ub = const.tile([P, KB, K, NU, MB, P], f16)

            def emit_U(u, mb):
                # 1-term rows on ACT (copy-scale); 3-term rows as DVE chains
                # (keeps the ACT and DVE streams both short at startup)
                nz = [kw for kw in range(K) if G_ROWS[u][kw] != 0.0]
                for kb in range(KB):
                    for kh in range(K):
                        srcs = [wfin[:, kb, kh * K + kw, mb, :] for kw in nz]
                        dst = Ub[:, kb, kh, u, mb, :]
                        if len(nz) == 1:
                            if mb == 0:
                                nc.scalar.activation(
                                    out=dst, in_=srcs[0], func=Act.Copy,
                                    scale=cu[:, u, nz[0] : nz[0] + 1],
                                )
                            else:
                                nc.vector.tensor_scalar_mul(
                                    out=dst, in0=srcs[0],
                                    scalar1=cu[:, u, nz[0] : nz[0] + 1],
                                )
                        else:
                            ta = utmp.tile([P, P], f32, name="ta")
                            nc.vector.tensor_scalar_mul(
                                out=ta, in0=srcs[0],
                                scalar1=cu[:, u, nz[0] : nz[0] + 1],
                            )
                            tb = utmp.tile([P, P], f32, name="tb")
                            nc.vector.scalar_tensor_tensor(
                                out=tb, in0=srcs[1],
                                scalar=cu[:, u, nz[1] : nz[1] + 1],
                                in1=ta, op0=Alu.mult, op1=Alu.add,
                            )
                            nc.vector.scalar_tensor_tensor(
                                out=dst, in0=srcs[2],
                                scalar=cu[:, u, nz[2] : nz[2] + 1],
                                in1=tb, op0=Alu.mult, op1=Alu.add,
                            )

            # piece-0 helpers first: conv u0 needs them with U[u0]
            emit_helpers(0, 0)
            emit_helpers(0, 1)
            # U built mb-major so chunk0-mb0's weights are ready first
            for mb in range(MB):
                for u in range(NU):
                    emit_U(u, mb)

            if stage == "wprep":
                ot = outp.tile([P, KB * K * NU * MB * P], f16)
                nc.vector.tensor_copy(
                    out=ot, in_=Ub[:].rearrange("p a b c d e -> p (a b c d e)")
                )
                nc.sync.dma_start(out=y_flat[0:P, 0 : KB * K * NU * MB * P], in_=ot)
                ot2 = outp.tile([P, K], f16)
                nc.vector.tensor_copy(out=ot2, in_=demod)
                nc.sync.dma_start(out=y_flat[0:P, 16000 : 16000 + K], in_=ot2)

            if stage == "full":
                # ---------- conv over 8 pieces ----------
                def emit_chunk(p, lr0, nrows):
                    r0 = p * PROWS + lr0
                    nf = nrows * W
                    ots = outp.tile([P, MB, CH_ROWS, W], f16, name="ot")
                    for mb in range(MB):
                        yt = outp.tile([P, CH_ROWS, W], f32, name="yt")
                        pt = psum.tile([P, NU, CH_ROWS, NT], f32, name="pc")
                        # u1/u2 first: the epilogue's ACT copies (which gate
                        # the psum release) start before u0/u3 finish
                        for u in (1, 2, 0, 3):
                            first = True
                            for kb in range(KB):
                                for kh in range(K):
                                    nc.tensor.matmul(
                                        pt[:, u, 0:nrows, :],
                                        Ub[:, kb, kh, u, mb, :],
                                        rhs_view(p, kb, u, lr0, kh, nrows),
                                        start=first,
                                        stop=(kb == KB - 1 and kh == K - 1),
                                    )
                                    first = False
                        # ---- epilogue ----
                        c1 = ctmp.tile([P, CH_ROWS, NT], f32, name="c1")
                        c2 = ctmp.tile([P, CH_ROWS, NT], f32, name="c2")
                        nc.scalar.activation(
                            out=c1[:, 0:nrows, :], in_=pt[:, 1, 0:nrows, :],
                            func=Act.Copy)
                        nc.scalar.activation(
                            out=c2[:, 0:nrows, :], in_=pt[:, 2, 0:nrows, :],
                            func=Act.Copy)
                        s01 = ctmp.tile([P, CH_ROWS, NT], f32, name="s01")
                        d12 = ctmp.tile([P, CH_ROWS, NT], f32, name="d12")
                        nc.vector.tensor_add(
                            s01[:, 0:nrows, :], c1[:, 0:nrows, :],
                            c2[:, 0:nrows, :])
                        nc.vector.tensor_sub(
                            d12[:, 0:nrows, :], c1[:, 0:nrows, :],
                            c2[:, 0:nrows, :])
                        nc.vector.tensor_add(
                            yt[:, 0:nrows, 0::2], s01[:, 0:nrows, :],
                            pt[:, 0, 0:nrows, :])
                        nc.vector.tensor_sub(
                            yt[:, 0:nrows, 1::2], d12[:, 0:nrows, :],
                            pt[:, 3, 0:nrows, :])
                        # leaky relu + 1/64 descale + f16 cast on ACT
                        nc.scalar.activation(
                            out=ots[:, mb, 0:nrows, :], in_=yt[:, 0:nrows, :],
                            func=Act.Prelu,
                            scale=1.0 / USCALE, alpha=0.2)
                    nc.sync.dma_start(
                        out=y_pmf[:, :, r0 * W : r0 * W + nf],
                        in_=ots[:, :, 0:nrows, :],
                    )

                for p in range(NP):
                    if p < NP - 1:
                        emit_chunk(p, 0, CH_ROWS)
                        # next-piece helpers here: their ttmp buffers (from
                        # piece p-1) are free, they don't head-of-line-block
                        # the c0 epilogues, and they finish well before the
                        # piece boundary
                        emit_helpers(p + 1, 0)
                        emit_helpers(p + 1, 1)
                        emit_chunk(p, CH_ROWS, CH_ROWS)
                    else:
                        # split the final chunk so the drain tail is short
                        emit_chunk(p, 0, CH_ROWS)
                        emit_chunk(p, CH_ROWS, CH_ROWS // 2)
                        emit_chunk(p, CH_ROWS + CH_ROWS // 2, CH_ROWS // 2)
    nc.compile()
    return nc


def _get_nc():
    if "nc" not in _CACHE:
        _CACHE["nc"] = _build()
    return _CACHE["nc"]


def prep_in_maps(input_vector, style_vector, weight):
    """Host-side staging: fp16 casts, per-core input dicts."""
    x16 = np.ascontiguousarray(input_vector, dtype=np.float16)
    w16 = np.ascontiguousarray(weight, dtype=np.float16)
    s32 = np.ascontiguousarray(style_vector, dtype=np.float32)
    return [
        {"x": x16[b], "style": s32[b : b + 1], "w": w16}
        for b in range(B)
    ]


def _get_runner():
    """Build (once) a reusable jitted shard_map runner over the 8 cores, so
    repeated kernel() calls skip re-tracing/lowering the bass module."""
    if "runner" in _CACHE:
        return _CACHE["runner"]

    import jax
    import concourse.bass2jax as b2j
    import concourse.mybir as mybir
    from jax.experimental.shard_map import shard_map
    from jax.sharding import Mesh, PartitionSpec

    nc = _get_nc()
    b2j.install_neuronx_cc_hook()

    partition_name = nc.partition_id_tensor.name if nc.partition_id_tensor else None
    in_names, out_names, out_avals, zero_outs = [], [], [], []
    for alloc in nc.m.functions[0].allocations:
        if not isinstance(alloc, mybir.MemoryLocationSet):
            continue
        name = alloc.memorylocations[0].name
        if alloc.kind == "ExternalInput":
            if name != partition_name:
                in_names.append(name)
        elif alloc.kind == "ExternalOutput":
            out_names.append(name)
            shape = tuple(alloc.tensor_shape)
            dtype = mybir.dt.np(alloc.dtype)
            out_avals.append(jax.core.ShapedArray(shape, dtype))
            zero_outs.append(np.zeros(shape, dtype))
    n_params = len(in_names)
    n_outs = len(out_avals)
    all_in_names = list(in_names) + list(out_names)
    if partition_name is not None:
        all_in_names.append(partition_name)

    def _body(*args):
        operands = list(args)
        if partition_name is not None:
            operands.append(b2j.partition_id_tensor())
        outs = b2j._bass_exec_p.bind(
            *operands,
            out_avals=tuple(out_avals),
            in_names=tuple(all_in_names),
            out_names=tuple(out_names),
            lowering_input_output_aliases=(),
            sim_require_finite=True,
            sim_require_nnan=True,
            nc=nc,
        )
        return tuple(outs)

    devices = jax.devices()[:B]
    mesh = Mesh(np.asarray(devices), ("core",))
    in_specs = (PartitionSpec("core"),) * (n_params + n_outs)
    out_specs = (PartitionSpec("core"),) * len(out_names)
    sharded = jax.jit(
        shard_map(_body, mesh=mesh, in_specs=in_specs, out_specs=out_specs,
                  check_rep=False),
        donate_argnums=tuple(range(n_params, n_params + n_outs)),
        keep_unused=True,
    )
    _CACHE["runner"] = (sharded, in_names, out_names, out_avals, zero_outs)
    return _CACHE["runner"]


def finish_out(y_stack):
    """Raw stacked per-core outputs (B, COUT, H, W) -> full f32 output."""
    return np.ascontiguousarray(y_stack).astype(np.float32)


def kernel(input_vector, style_vector, weight):
    in_maps = prep_in_maps(input_vector, style_vector, weight)
    try:
        sharded, in_names, out_names, out_avals, zero_outs = _get_runner()
        concat_in = [
            np.concatenate([in_maps[c][nm] for c in range(B)], axis=0)
            for nm in in_names
        ]
        zeros = [
            np.zeros((B * z.shape[0], *z.shape[1:]), z.dtype) for z in zero_outs
        ]
        out_arrs = sharded(*concat_in, *zeros)
        yi = out_names.index("y")
        out = np.asarray(out_arrs[yi]).reshape(B, *out_avals[yi].shape)
    except Exception:
        # fallback: the one-shot path (slower per call, same result)
        from concourse.bass_utils import run_bass_kernel_spmd

        _CACHE.pop("runner", None)
        res = run_bass_kernel_spmd(_get_nc(), in_maps, core_ids=list(range(B)))
        out = np.stack([res.results[b]["y"] for b in range(B)], axis=0)
    return out.astype(np.float32)
